# revision 26
# baseline (speedup 1.0000x reference)
"""DeformConvBlock Trainium2 kernel (data-parallel over batch across 8 cores).

Per-core (1 image, C=128, O=128, H=W=80, 3x3):
  1. offset = conv3x3(x, w_off) + b_off            (PE fp32 im2col GEMM)
  2. bilinear deform sampling via affine-basis identity:
       sample = P0[q] + dy*P1[q] + dx*P2[q] + dy*dx*P3[q],
     q = (floor(py), floor(px)) in an 8-padded image; P0..P3 = x and its
     v/h/cross shifted differences. One 1KB gather row per (tap,pixel).
  3. dma_gather 57.6K rows from DRAM [q, 4*C] bf16 -> (tap,pixel) rows on
     partitions; 3 scalar_tensor_tensor ops with per-partition dy/dx give
     the deformed im2col tile v[(k,p), c].
  4. PE transpose v tiles -> [c, p]; bf16 GEMM with w; + bias -> out.
"""

import contextlib
import os
BISECT = os.environ.get('KBISECT', '')
import numpy as np
import ml_dtypes

import jax
# Persistent compilation cache: run_bass_kernel_spmd builds a fresh jax.jit
# closure per call, so without this every call pays a full XLA recompile.
try:
    jax.config.update("jax_compilation_cache_dir", "/tmp/jax_comp_cache")
    jax.config.update("jax_persistent_cache_min_compile_time_secs", 0.0)
    jax.config.update("jax_persistent_cache_min_entry_size_bytes", -1)
except Exception:
    pass

import concourse.bass as bass
import concourse.tile as tile
from concourse import bacc, mybir
from concourse import bass_utils

F32 = mybir.dt.float32
BF16 = mybir.dt.bfloat16
I8 = mybir.dt.int8
I16 = mybir.dt.int16
I32 = mybir.dt.int32
A = mybir.AluOpType

N, C, O, H, W = 8, 128, 128, 80, 80
K = 9
PAD = 8
WP = H + 2 * PAD          # 96
QP = WP * WP              # 9216
HWi = H * W               # 6400
NT = HWi // 128           # 50 pixel tiles
NTT = NT * K              # 450 gather tiles
NJ = NTT * 128            # 57600 gather rows
CLAMP_MAX = float(WP - 2)


def build_kernel(num_devices=N, debug=False):
    nc = bacc.Bacc("TRN2", target_bir_lowering=False, debug=False,
                   num_devices=num_devices)

    x_in = nc.dram_tensor("x", [C, HWi], BF16, kind="ExternalInput").ap()
    w_off_t = nc.dram_tensor("w_off_t", [C, K * 18], F32, kind="ExternalInput").ap()
    w_t = nc.dram_tensor("w_t", [C, K * O], BF16, kind="ExternalInput").ap()
    b_in = nc.dram_tensor("b", [O, 1], F32, kind="ExternalInput").ap()
    # per-pixel-tile base coords: cols [0,NT)=py+PAD, [NT,2NT)=px+PAD
    pypx_in = nc.dram_tensor("pypx", [128, 2 * NT], F32, kind="ExternalInput").ap()
    # per-offset-channel constant kh/kw + b_off, replicated across partitions
    kb_in = nc.dram_tensor("kb", [128, 18], F32, kind="ExternalInput").ap()

    # int8 output + per-(row, pixel-tile) scale: y = yq * ysc
    y_out = nc.dram_tensor("y", [O, HWi], I8, kind="ExternalOutput").ap()
    ysc_out = nc.dram_tensor("ysc", [O, NT], F32, kind="ExternalOutput").ap()
    dbg = {}
    if debug:
        for nm, shp, dt in (("off", [18, HWi], F32), ("idx", [C, NTT], I16),
                            ("dy", [C, NTT], F32), ("dx", [C, NTT], F32),
                            ("idxw", [C, NJ // 16], I16),
                            ("p4", [QP, 4 * C], BF16)):
            dbg[nm] = nc.dram_tensor("d_" + nm, shp, dt, kind="ExternalOutput").ap()

    p4_dram = nc.dram_tensor("p4_dram", [QP, 4 * C], BF16, kind="Internal").ap()
    idx_dram = nc.dram_tensor("idx_dram", [C, NTT], I16, kind="Internal").ap()

    with tile.TileContext(nc) as tc:
        with contextlib.ExitStack() as ctx:
            _body(ctx, tc, nc, x_in, w_off_t, w_t, b_in, pypx_in,
                  kb_in, y_out, ysc_out, p4_dram, idx_dram, dbg)
    nc.compile()
    return nc


def _body(ctx, tc, nc, x_in, w_off_t, w_t, b_in, pypx_in,
          kb_in, y_out, ysc_out, p4_dram, idx_dram, dbg):
    const = ctx.enter_context(tc.tile_pool(name="const", bufs=1))
    pers = ctx.enter_context(tc.tile_pool(name="pers", bufs=1))

    # ---- constants ----
    iid = const.tile([128, 128], I32)
    nc.gpsimd.iota(iid[:], pattern=[[-1, 128]], base=0, channel_multiplier=1)
    ident = const.tile([128, 128], F32)
    nc.vector.tensor_scalar(ident[:], iid[:], 0, None, op0=A.is_equal)
    identb = const.tile([128, 128], BF16)
    nc.scalar.copy(identb[:], ident[:])
    bias = const.tile([O, 1], F32)
    nc.sync.dma_start(bias[:], b_in)
    pypx = const.tile([128, 2 * NT], F32)
    nc.sync.dma_start(pypx[:], pypx_in)
    kbB = const.tile([128, 18], F32)
    nc.sync.dma_start(kbB[:], kb_in)
    woff = const.tile([C, K * 18], F32)
    nc.sync.dma_start(woff[:], w_off_t)
    wmat = const.tile([C, K * O], BF16)
    nc.sync.dma_start(wmat[:], w_t)

    # ---- persistent SBUF ----
    off_sb = pers.tile([18, HWi], F32)
    ysc_sb = pers.tile([O, NT], F32)
    idxS = pers.tile([C, NTT], I16)
    dyS = pers.tile([C, NTT], F32)
    dxS = pers.tile([C, NTT], F32)
    idxW = pers.tile([C, NJ // 16], I16)

    # ================= phase 1: load, offset conv, planes, P4 =================
    with tc.tile_pool(name="ph1", bufs=1) as ph1, \
         tc.tile_pool(name="ph1s", bufs=3) as ph1s, \
         tc.tile_pool(name="ps_off", bufs=2, space="PSUM") as ps_off, \
         tc.tile_pool(name="ps_tp1", bufs=3, space="PSUM") as ps_tp:
        xp = ph1.tile([C, QP], BF16)
        nc.gpsimd.memset(xp[:], 0.0)
        xp3 = xp[:].rearrange("c (h w) -> c h w", h=WP)
        nc.sync.dma_start(xp3[:, PAD:PAD + H, PAD:PAD + W],
                          x_in.rearrange("c (h w) -> c h w", h=H))
        # fp32 upcast for the offset conv (keeps offset precision)
        xf = ph1.tile([C, QP], F32)
        nc.scalar.copy(xf[:], xp[:])
        xf3 = xf[:].rearrange("c (h w) -> c h w", h=WP)

        # offset conv (fp32), chunks of 6 output rows (N=480)
        CH = 6
        for yc in range(0, H, CH):
            rows = min(CH, H - yc)
            po = ps_off.tile([18, CH * W], F32, tag="po")
            for k in range(K):
                kh, kw = divmod(k, 3)
                rhs = xf3[:, (yc + kh - 1 + PAD):(yc + kh - 1 + PAD) + rows,
                          (kw - 1 + PAD):(kw - 1 + PAD) + W]
                nc.tensor.matmul(po[:, :rows * W],
                                 woff[:, k * 18:(k + 1) * 18], rhs,
                                 start=(k == 0), stop=(k == K - 1))
            nc.scalar.copy(off_sb[:, yc * W:(yc + rows) * W], po[:, :rows * W])
        if dbg:
            nc.sync.dma_start(dbg["off"], off_sb[:])

        # bf16 planes
        xb = xp
        d1 = ph1.tile([C, QP], BF16)
        nc.gpsimd.memset(d1[:, QP - WP:], 0.0)
        nc.vector.tensor_tensor(d1[:, :QP - WP], xb[:, WP:], xb[:, :QP - WP], op=A.subtract)
        d2 = ph1.tile([C, QP], BF16)
        nc.gpsimd.memset(d2[:, QP - 1:], 0.0)
        nc.vector.tensor_tensor(d2[:, :QP - 1], xb[:, 1:], xb[:, :QP - 1], op=A.subtract)
        d3 = ph1.tile([C, QP], BF16)
        nc.gpsimd.memset(d3[:, QP - WP:], 0.0)
        nc.vector.tensor_tensor(d3[:, :QP - WP], d2[:, WP:], d2[:, :QP - WP], op=A.subtract)
        planes = [xb, d1, d2, d3]

        # zero all of P4 first (pads), then overwrite the active interior
        zbuf = ph1.tile([128, 8192], BF16)
        nc.gpsimd.memset(zbuf[:], 0.0)
        ZR = 2048  # rows per zero DMA (keeps every AP dim < 2^16)
        for r0 in range(0, QP, ZR):
            rows = min(ZR, QP - r0)
            nc.sync.dma_start(p4_dram[r0:r0 + rows, :],
                              zbuf[:, 0:rows * 512 // 128])

        # active region: rows/cols [PAD-1, PAD+H) of the padded image
        WA = W + 1  # 81
        for y in range(-1, H):
            qp0 = (y + PAD) * WP + (PAD - 1)
            stg = ph1s.tile([WA, 4 * C], BF16, tag="stg")
            for pi, pl in enumerate(planes):
                tp = ps_tp.tile([WA, 128], BF16, tag="tpp")
                nc.tensor.transpose(tp[:], pl[:, qp0:qp0 + WA], identb[:])
                nc.scalar.copy(stg[:, pi * C:(pi + 1) * C], tp[:])
            nc.sync.dma_start(p4_dram[qp0:qp0 + WA, :], stg[:])

    if dbg:
        nc.sync.dma_start(dbg["p4"], p4_dram)

    # ================= phase 2: maps =================
    with tc.tile_pool(name="ph2s", bufs=3) as sm, \
         tc.tile_pool(name="ps_tp2", bufs=2, space="PSUM") as ps_tp:
        for t in range(NT):
            offT_ps = ps_tp.tile([128, 18], F32, tag="offT")
            nc.tensor.transpose(offT_ps[:], off_sb[:, t * 128:(t + 1) * 128],
                                ident[0:18, 0:18])
            q = sm.tile([128, 18], F32, tag="mq")
            nc.scalar.copy(q[:], offT_ps[:])
            nc.vector.tensor_tensor(q[:], q[:], kbB[:], op=A.add)
            q2v = q[:].rearrange("p (k two) -> p k two", two=2)
            nc.vector.tensor_scalar_add(q2v[:, :, 0], q2v[:, :, 0],
                                        pypx[:, t:t + 1])
            nc.vector.tensor_scalar_add(q2v[:, :, 1], q2v[:, :, 1],
                                        pypx[:, NT + t:NT + t + 1])
            nc.vector.tensor_scalar_min(q[:], q[:], CLAMP_MAX)
            nc.vector.tensor_scalar_max(q[:], q[:], 0.0)
            qi = sm.tile([128, 18], I32, tag="mqi")
            nc.vector.tensor_copy(qi[:], q[:])          # rne
            qr = sm.tile([128, 18], F32, tag="mqr")
            nc.vector.tensor_copy(qr[:], qi[:])
            m = sm.tile([128, 18], F32, tag="mm")
            nc.vector.tensor_tensor(m[:], qr[:], q[:], op=A.is_gt)
            fl = sm.tile([128, 18], F32, tag="mfl")
            nc.vector.tensor_tensor(fl[:], qr[:], m[:], op=A.subtract)
            dd = sm.tile([128, 18], F32, tag="mdd")
            nc.vector.tensor_tensor(dd[:], q[:], fl[:], op=A.subtract)
            fl2 = fl[:].rearrange("p (k two) -> p k two", two=2)
            dd2 = dd[:].rearrange("p (k two) -> p k two", two=2)
            fidx = sm.tile([128, 9], F32, tag="mfi")
            nc.vector.scalar_tensor_tensor(fidx[:], fl2[:, :, 0], float(WP),
                                           fl2[:, :, 1], op0=A.mult, op1=A.add)
            nc.vector.tensor_copy(idxS[:, t * K:(t + 1) * K], fidx[:])
            nc.vector.tensor_copy(dyS[:, t * K:(t + 1) * K], dd2[:, :, 0])
            nc.vector.tensor_copy(dxS[:, t * K:(t + 1) * K], dd2[:, :, 1])

        # idx wrap: j = T*128+pp -> wrapped[pp%16, 8T + pp//16]
        if BISECT == 'B':
            nc.gpsimd.memset(idxW[:], 0)
        else:
            nc.sync.dma_start(idx_dram, idxS[:])
            w1 = sm.tile([16, 8 * NTT], I16, tag="w1")
            src2 = idx_dram.rearrange("(u r) t -> r u t", u=8)
            nc.sync.dma_start(w1[:].rearrange("r (u t) -> r u t", u=8), src2)
            w1v = w1[:].rearrange("r (u t) -> r t u", u=8)
            nc.vector.tensor_copy(idxW[0:16, :].rearrange("r (t u) -> r t u", u=8), w1v)
            for g in range(1, 8):
                nc.sync.dma_start(idxW[16 * g:16 * (g + 1), :], idxW[0:16, :])
    if dbg:
        nc.sync.dma_start(dbg["idx"], idxS[:])
        nc.sync.dma_start(dbg["dy"], dyS[:])
        nc.sync.dma_start(dbg["dx"], dxS[:])
        nc.sync.dma_start(dbg["idxw"], idxW[:])

    # ================= phase 3: gather + combine + GEMM =================
    if BISECT == 'A':
        with tc.tile_pool(name="za", bufs=1) as za:
            zy = za.tile([O, 400], I8)
            nc.gpsimd.memset(zy[:], 0)
            for t0 in range(0, HWi, 400):
                nc.sync.dma_start(y_out[:, t0:t0 + 400], zy[:])
            zs = za.tile([O, NT], F32)
            nc.gpsimd.memset(zs[:], 0.0)
            nc.sync.dma_start(ysc_out, zs[:])
        return
    TCH = 2
    with tc.tile_pool(name="gpool", bufs=3) as gpool, \
         tc.tile_pool(name="vpool", bufs=4) as vpool, \
         tc.tile_pool(name="opool", bufs=3) as opool, \
         tc.tile_pool(name="ps_out", bufs=2, space="PSUM") as ps_out, \
         tc.tile_pool(name="ps_tp3", bufs=3, space="PSUM") as ps_tp:
        for tc0 in range(0, NT, TCH):
            nt = min(TCH, NT - tc0)
            nidx = nt * K * 128
            gt = gpool.tile([128, TCH * K, 4 * C], BF16, tag="gather")
            c0 = tc0 * K * 8
            nc.gpsimd.dma_gather(gt[:, :nt * K, :], p4_dram,
                                 idxW[:, c0:c0 + nidx // 16],
                                 num_idxs=nidx, num_idxs_reg=nidx, elem_size=4 * C,
                                 single_packet=False)
            for ti in range(nt):
                t = tc0 + ti
                out_ps = ps_out.tile([O, 128], F32, tag="ops")
                for k in range(K):
                    T = t * K + k
                    g = gt[:, ti * K + k, :]
                    s1 = vpool.tile([128, C], BF16, tag="s1")
                    nc.vector.scalar_tensor_tensor(s1[:], g[:, C:2 * C],
                                                   dyS[:, T:T + 1], g[:, 0:C],
                                                   op0=A.mult, op1=A.add)
                    s2 = vpool.tile([128, C], BF16, tag="s2")
                    nc.vector.scalar_tensor_tensor(s2[:], g[:, 3 * C:4 * C],
                                                   dyS[:, T:T + 1], g[:, 2 * C:3 * C],
                                                   op0=A.mult, op1=A.add)
                    v = vpool.tile([128, C], BF16, tag="v")
                    nc.vector.scalar_tensor_tensor(v[:], s2[:], dxS[:, T:T + 1], s1[:],
                                                   op0=A.mult, op1=A.add)
                    vT_ps = ps_tp.tile([C, 128], BF16, tag="vT")
                    nc.tensor.transpose(vT_ps[:], v[:], identb[:])
                    vT = vpool.tile([C, 128], BF16, tag="vTs")
                    nc.scalar.copy(vT[:], vT_ps[:])
                    nc.tensor.matmul(out_ps[:], wmat[:, k * O:(k + 1) * O], vT[:],
                                     start=(k == 0), stop=(k == K - 1))
                ot = opool.tile([O, 128], F32, tag="ot")
                nc.vector.tensor_scalar_add(ot[:], out_ps[:], bias[:])
                s = opool.tile([O, 1], F32, tag="sc")
                nc.vector.tensor_reduce(s[:], ot[:], axis=mybir.AxisListType.X,
                                        op=A.max, apply_absolute_value=True)
                nc.vector.tensor_scalar_max(s[:], s[:], 1e-30)
                rs = opool.tile([O, 1], F32, tag="rs")
                nc.vector.reciprocal(rs[:], s[:])
                qt = opool.tile([O, 128], I8, tag="qt")
                nc.vector.tensor_scalar(qt[:], ot[:], rs[:], 127.0,
                                        op0=A.mult, op1=A.mult)
                nc.sync.dma_start(y_out[:, t * 128:(t + 1) * 128], qt[:])
                nc.vector.tensor_scalar_mul(ysc_sb[:, t:t + 1], s[:], 1.0 / 127.0)
        nc.sync.dma_start(ysc_out, ysc_sb[:])


# ================= host side =================

def _prep_inputs(x, w_off, b_off, w, b):
    # [C, K*18]: col k*18+e = w_off[e, c, k]
    wofft = np.ascontiguousarray(
        w_off.reshape(18, C, K).transpose(1, 2, 0).reshape(C, K * 18)).astype(np.float32)
    wt = np.ascontiguousarray(
        w.reshape(O, C, K).transpose(1, 2, 0).reshape(C, K * O)).astype(ml_dtypes.bfloat16)
    p = np.arange(HWi)
    py, px = p // W, p % W
    kh = np.arange(K) // 3 - 1
    kw = np.arange(K) % 3 - 1
    # pypx[pp, t] = py of pixel t*128+pp (+PAD); cols NT.. hold px
    pypx = np.empty((128, 2 * NT), np.float32)
    pypx[:, :NT] = (py + PAD).reshape(NT, 128).T
    pypx[:, NT:] = (px + PAD).reshape(NT, 128).T
    kb = np.zeros((18,), np.float32)
    kb[0::2] = kh
    kb[1::2] = kw
    kb += b_off.reshape(18)
    kbB = np.ascontiguousarray(np.broadcast_to(kb, (128, 18)))
    shared = {
        "w_off_t": wofft,
        "w_t": wt,
        "b": np.ascontiguousarray(b.reshape(O, 1)).astype(np.float32),
        "pypx": pypx,
        "kb": kbB,
    }
    return [dict(shared, x=np.ascontiguousarray(x[n].reshape(C, HWi)).astype(ml_dtypes.bfloat16))
            for n in range(x.shape[0])]


_CACHED = {}


def _get_nc(num_devices=N, debug=False):
    key = (num_devices, debug)
    if key not in _CACHED:
        _CACHED[key] = build_kernel(num_devices=num_devices, debug=debug)
    return _CACHED[key]


def kernel(x, w_off, b_off, w, b):
    x = np.asarray(x, np.float32)
    nc = _get_nc()
    core_ins = _prep_inputs(x, np.asarray(w_off, np.float32),
                            np.asarray(b_off, np.float32),
                            np.asarray(w, np.float32), np.asarray(b, np.float32))
    res = bass_utils.run_bass_kernel_spmd(nc, core_ins, core_ids=list(range(N)))
    out = np.empty((N, O, H, W), np.float32)
    for n in range(N):
        q = res.results[n]["y"].reshape(O, NT, 128).astype(np.float32)
        sc = res.results[n]["ysc"].reshape(O, NT, 1).astype(np.float32)
        out[n] = (q * sc).reshape(O, H, W)
    return out



# revision 31
# speedup vs baseline: 1.1940x; 1.1940x over previous
"""DeformConvBlock Trainium2 kernel (data-parallel over batch across 8 cores).

Per-core (1 image, C=128, O=128, H=W=80, 3x3):
  1. offset = conv3x3(x, w_off) + b_off            (PE fp32 im2col GEMM)
  2. bilinear deform sampling via affine-basis identity:
       sample = P0[q] + dy*P1[q] + dx*P2[q] + dy*dx*P3[q],
     q = (floor(py), floor(px)) in an 8-padded image; P0..P3 = x and its
     v/h/cross shifted differences. One 1KB gather row per (tap,pixel).
  3. dma_gather 57.6K rows from DRAM [q, 4*C] bf16 -> (tap,pixel) rows on
     partitions; 3 scalar_tensor_tensor ops with per-partition dy/dx give
     the deformed im2col tile v[(k,p), c].
  4. PE transpose v tiles -> [c, p]; bf16 GEMM with w; + bias -> out.
"""

import contextlib
import os
BISECT = os.environ.get('KBISECT', '')
import numpy as np
import ml_dtypes

import jax
# Persistent compilation cache: run_bass_kernel_spmd builds a fresh jax.jit
# closure per call, so without this every call pays a full XLA recompile.
try:
    jax.config.update("jax_compilation_cache_dir", "/tmp/jax_comp_cache")
    jax.config.update("jax_persistent_cache_min_compile_time_secs", 0.0)
    jax.config.update("jax_persistent_cache_min_entry_size_bytes", -1)
except Exception:
    pass

import concourse.bass as bass
import concourse.tile as tile
from concourse import bacc, mybir
from concourse import bass_utils

F32 = mybir.dt.float32
BF16 = mybir.dt.bfloat16
I8 = mybir.dt.int8
I16 = mybir.dt.int16
I32 = mybir.dt.int32
A = mybir.AluOpType

N, C, O, H, W = 8, 128, 128, 80, 80
K = 9
# packed replicated constants (uploaded 1/N per core, AllGathered on device)
NB_WT = C * K * O * 2        # w_t   bf16 [C, K*O]
NB_WOFF = C * K * 18 * 4     # woff  f32  [C, K*18]
NB_PYPX = 128 * 100 * 4      # pypx  f32  [128, 2*NT]
NB_KB = 128 * 18 * 4         # kb    f32  [128, 18]
NB_B = 128 * 4               # b     f32  [O, 1]
S_PACK = NB_WT + NB_WOFF + NB_PYPX + NB_KB + NB_B
S8 = S_PACK // N
PAD = 8
WP = H + 2 * PAD          # 96
QP = WP * WP              # 9216
HWi = H * W               # 6400
NT = HWi // 128           # 50 pixel tiles
NTT = NT * K              # 450 gather tiles
NJ = NTT * 128            # 57600 gather rows
CLAMP_MAX = float(WP - 2)


def build_kernel(num_devices=N, debug=False):
    nc = bacc.Bacc("TRN2", target_bir_lowering=False, debug=False,
                   num_devices=num_devices)

    assert num_devices == N
    x_in = nc.dram_tensor("x", [C, HWi], BF16, kind="ExternalInput").ap()
    cpack_in = nc.dram_tensor("cpack", [1, S8], mybir.dt.uint8,
                              kind="ExternalInput").ap()

    # int8 output + per-(row, pixel-tile) scale: y = yq * ysc
    y_out = nc.dram_tensor("y", [O, HWi], I8, kind="ExternalOutput").ap()
    ysc_out = nc.dram_tensor("ysc", [O, NT], F32, kind="ExternalOutput").ap()
    dbg = {}
    if debug:
        for nm, shp, dt in (("off", [18, HWi], F32), ("idx", [C, NTT], I16),
                            ("dy", [C, NTT], F32), ("dx", [C, NTT], F32),
                            ("idxw", [C, NJ // 16], I16),
                            ("p4", [QP, 4 * C], BF16)):
            dbg[nm] = nc.dram_tensor("d_" + nm, shp, dt, kind="ExternalOutput").ap()

    p4_dram = nc.dram_tensor("p4_dram", [QP, 4 * C], BF16, kind="Internal").ap()
    idx_dram = nc.dram_tensor("idx_dram", [C, NTT], I16, kind="Internal").ap()

    with tile.TileContext(nc) as tc:
        with contextlib.ExitStack() as ctx:
            _body(ctx, tc, nc, x_in, cpack_in, y_out, ysc_out,
                  p4_dram, idx_dram, dbg)
    nc.compile()
    return nc


def _body(ctx, tc, nc, x_in, cpack_in, y_out, ysc_out,
          p4_dram, idx_dram, dbg):
    const = ctx.enter_context(tc.tile_pool(name="const", bufs=1))
    pers = ctx.enter_context(tc.tile_pool(name="pers", bufs=1))

    # ---- constants: AllGather the packed shard, then unpack ----
    ccd = ctx.enter_context(tc.tile_pool(name="ccd", bufs=1, space="DRAM"))
    cin = ccd.tile([1, S8], mybir.dt.uint8)
    cout = ccd.tile([1, S_PACK], mybir.dt.uint8)
    nc.gpsimd.dma_start(cin[:], cpack_in)
    nc.gpsimd.collective_compute(
        "AllGather", A.bypass, replica_groups=[list(range(N))],
        ins=[cin.opt()], outs=[cout.opt()])

    def unpack(off, nbytes, dt, parts):
        return cout[0, off:off + nbytes].bitcast(dt).rearrange(
            "(c f) -> c f", c=parts)

    iid = const.tile([128, 128], I32)
    nc.gpsimd.iota(iid[:], pattern=[[-1, 128]], base=0, channel_multiplier=1)
    ident = const.tile([128, 128], F32)
    nc.vector.tensor_scalar(ident[:], iid[:], 0, None, op0=A.is_equal)
    identb = const.tile([128, 128], BF16)
    nc.scalar.copy(identb[:], ident[:])
    o_wt, o_woff = 0, NB_WT
    o_pypx = o_woff + NB_WOFF
    o_kb = o_pypx + NB_PYPX
    o_b = o_kb + NB_KB
    wmat = const.tile([C, K * O], BF16)
    nc.sync.dma_start(wmat[:], unpack(o_wt, NB_WT, BF16, C))
    woff = const.tile([C, K * 18], F32)
    nc.sync.dma_start(woff[:], unpack(o_woff, NB_WOFF, F32, C))
    pypx = const.tile([128, 2 * NT], F32)
    nc.sync.dma_start(pypx[:], unpack(o_pypx, NB_PYPX, F32, 128))
    kbB = const.tile([128, 18], F32)
    nc.sync.dma_start(kbB[:], unpack(o_kb, NB_KB, F32, 128))
    bias = const.tile([O, 1], F32)
    nc.sync.dma_start(bias[:], unpack(o_b, NB_B, F32, O))

    # ---- persistent SBUF ----
    off_sb = pers.tile([18, HWi], F32)
    ysc_sb = pers.tile([O, NT], F32)
    idxS = pers.tile([C, NTT], I16)
    dyS = pers.tile([C, NTT], F32)
    dxS = pers.tile([C, NTT], F32)
    idxW = pers.tile([C, NJ // 16], I16)

    # ================= phase 1: load, offset conv, planes, P4 =================
    with tc.tile_pool(name="ph1", bufs=1) as ph1, \
         tc.tile_pool(name="ph1s", bufs=3) as ph1s, \
         tc.tile_pool(name="ps_off", bufs=2, space="PSUM") as ps_off, \
         tc.tile_pool(name="ps_tp1", bufs=3, space="PSUM") as ps_tp:
        xp = ph1.tile([C, QP], BF16)
        nc.gpsimd.memset(xp[:], 0.0)
        xp3 = xp[:].rearrange("c (h w) -> c h w", h=WP)
        nc.sync.dma_start(xp3[:, PAD:PAD + H, PAD:PAD + W],
                          x_in.rearrange("c (h w) -> c h w", h=H))
        # fp32 upcast for the offset conv (keeps offset precision)
        xf = ph1.tile([C, QP], F32)
        nc.scalar.copy(xf[:], xp[:])
        xf3 = xf[:].rearrange("c (h w) -> c h w", h=WP)

        # offset conv (fp32), chunks of 6 output rows (N=480)
        CH = 6
        for yc in range(0, H, CH):
            rows = min(CH, H - yc)
            po = ps_off.tile([18, CH * W], F32, tag="po")
            for k in range(K):
                kh, kw = divmod(k, 3)
                rhs = xf3[:, (yc + kh - 1 + PAD):(yc + kh - 1 + PAD) + rows,
                          (kw - 1 + PAD):(kw - 1 + PAD) + W]
                nc.tensor.matmul(po[:, :rows * W],
                                 woff[:, k * 18:(k + 1) * 18], rhs,
                                 start=(k == 0), stop=(k == K - 1))
            nc.scalar.copy(off_sb[:, yc * W:(yc + rows) * W], po[:, :rows * W])
        if dbg:
            nc.sync.dma_start(dbg["off"], off_sb[:])

        # bf16 planes
        xb = xp
        d1 = ph1.tile([C, QP], BF16)
        nc.gpsimd.memset(d1[:, QP - WP:], 0.0)
        nc.vector.tensor_tensor(d1[:, :QP - WP], xb[:, WP:], xb[:, :QP - WP], op=A.subtract)
        d2 = ph1.tile([C, QP], BF16)
        nc.gpsimd.memset(d2[:, QP - 1:], 0.0)
        nc.vector.tensor_tensor(d2[:, :QP - 1], xb[:, 1:], xb[:, :QP - 1], op=A.subtract)
        d3 = ph1.tile([C, QP], BF16)
        nc.gpsimd.memset(d3[:, QP - WP:], 0.0)
        nc.vector.tensor_tensor(d3[:, :QP - WP], d2[:, WP:], d2[:, :QP - WP], op=A.subtract)
        planes = [xb, d1, d2, d3]

        # zero all of P4 first (pads), then overwrite the active interior
        zbuf = ph1.tile([128, 8192], BF16)
        nc.gpsimd.memset(zbuf[:], 0.0)
        ZR = 2048  # rows per zero DMA (keeps every AP dim < 2^16)
        for r0 in range(0, QP, ZR):
            rows = min(ZR, QP - r0)
            nc.sync.dma_start(p4_dram[r0:r0 + rows, :],
                              zbuf[:, 0:rows * 512 // 128])

        # active region: rows/cols [PAD-1, PAD+H) of the padded image
        WA = W + 1  # 81
        for y in range(-1, H):
            qp0 = (y + PAD) * WP + (PAD - 1)
            stg = ph1s.tile([WA, 4 * C], BF16, tag="stg")
            for pi, pl in enumerate(planes):
                tp = ps_tp.tile([WA, 128], BF16, tag="tpp")
                nc.tensor.transpose(tp[:], pl[:, qp0:qp0 + WA], identb[:])
                nc.scalar.copy(stg[:, pi * C:(pi + 1) * C], tp[:])
            nc.sync.dma_start(p4_dram[qp0:qp0 + WA, :], stg[:])

    if dbg:
        nc.sync.dma_start(dbg["p4"], p4_dram)

    # ================= phase 2: maps =================
    with tc.tile_pool(name="ph2s", bufs=3) as sm, \
         tc.tile_pool(name="ps_tp2", bufs=2, space="PSUM") as ps_tp:
        for t in range(NT):
            offT_ps = ps_tp.tile([128, 18], F32, tag="offT")
            nc.tensor.transpose(offT_ps[:], off_sb[:, t * 128:(t + 1) * 128],
                                ident[0:18, 0:18])
            q = sm.tile([128, 18], F32, tag="mq")
            nc.scalar.copy(q[:], offT_ps[:])
            nc.vector.tensor_tensor(q[:], q[:], kbB[:], op=A.add)
            q2v = q[:].rearrange("p (k two) -> p k two", two=2)
            nc.vector.tensor_scalar_add(q2v[:, :, 0], q2v[:, :, 0],
                                        pypx[:, t:t + 1])
            nc.vector.tensor_scalar_add(q2v[:, :, 1], q2v[:, :, 1],
                                        pypx[:, NT + t:NT + t + 1])
            nc.vector.tensor_scalar_min(q[:], q[:], CLAMP_MAX)
            nc.vector.tensor_scalar_max(q[:], q[:], 0.0)
            qi = sm.tile([128, 18], I32, tag="mqi")
            nc.vector.tensor_copy(qi[:], q[:])          # rne
            qr = sm.tile([128, 18], F32, tag="mqr")
            nc.vector.tensor_copy(qr[:], qi[:])
            m = sm.tile([128, 18], F32, tag="mm")
            nc.vector.tensor_tensor(m[:], qr[:], q[:], op=A.is_gt)
            fl = sm.tile([128, 18], F32, tag="mfl")
            nc.vector.tensor_tensor(fl[:], qr[:], m[:], op=A.subtract)
            dd = sm.tile([128, 18], F32, tag="mdd")
            nc.vector.tensor_tensor(dd[:], q[:], fl[:], op=A.subtract)
            fl2 = fl[:].rearrange("p (k two) -> p k two", two=2)
            dd2 = dd[:].rearrange("p (k two) -> p k two", two=2)
            fidx = sm.tile([128, 9], F32, tag="mfi")
            nc.vector.scalar_tensor_tensor(fidx[:], fl2[:, :, 0], float(WP),
                                           fl2[:, :, 1], op0=A.mult, op1=A.add)
            nc.vector.tensor_copy(idxS[:, t * K:(t + 1) * K], fidx[:])
            nc.vector.tensor_copy(dyS[:, t * K:(t + 1) * K], dd2[:, :, 0])
            nc.vector.tensor_copy(dxS[:, t * K:(t + 1) * K], dd2[:, :, 1])

        # idx wrap: j = T*128+pp -> wrapped[pp%16, 8T + pp//16]
        if BISECT == 'B':
            nc.gpsimd.memset(idxW[:], 0)
        else:
            nc.sync.dma_start(idx_dram, idxS[:])
            w1 = sm.tile([16, 8 * NTT], I16, tag="w1")
            src2 = idx_dram.rearrange("(u r) t -> r u t", u=8)
            nc.sync.dma_start(w1[:].rearrange("r (u t) -> r u t", u=8), src2)
            w1v = w1[:].rearrange("r (u t) -> r t u", u=8)
            nc.vector.tensor_copy(idxW[0:16, :].rearrange("r (t u) -> r t u", u=8), w1v)
            for g in range(1, 8):
                nc.sync.dma_start(idxW[16 * g:16 * (g + 1), :], idxW[0:16, :])
    if dbg:
        nc.sync.dma_start(dbg["idx"], idxS[:])
        nc.sync.dma_start(dbg["dy"], dyS[:])
        nc.sync.dma_start(dbg["dx"], dxS[:])
        nc.sync.dma_start(dbg["idxw"], idxW[:])

    # ================= phase 3: gather + combine + GEMM =================
    if BISECT == 'A':
        with tc.tile_pool(name="za", bufs=1) as za:
            zy = za.tile([O, 400], I8)
            nc.gpsimd.memset(zy[:], 0)
            for t0 in range(0, HWi, 400):
                nc.sync.dma_start(y_out[:, t0:t0 + 400], zy[:])
            zs = za.tile([O, NT], F32)
            nc.gpsimd.memset(zs[:], 0.0)
            nc.sync.dma_start(ysc_out, zs[:])
        return
    TCH = 2
    with tc.tile_pool(name="gpool", bufs=3) as gpool, \
         tc.tile_pool(name="vpool", bufs=4) as vpool, \
         tc.tile_pool(name="opool", bufs=3) as opool, \
         tc.tile_pool(name="ps_out", bufs=2, space="PSUM") as ps_out, \
         tc.tile_pool(name="ps_tp3", bufs=3, space="PSUM") as ps_tp:
        for tc0 in range(0, NT, TCH):
            nt = min(TCH, NT - tc0)
            nidx = nt * K * 128
            gt = gpool.tile([128, TCH * K, 4 * C], BF16, tag="gather")
            c0 = tc0 * K * 8
            nc.gpsimd.dma_gather(gt[:, :nt * K, :], p4_dram,
                                 idxW[:, c0:c0 + nidx // 16],
                                 num_idxs=nidx, num_idxs_reg=nidx, elem_size=4 * C,
                                 single_packet=False)
            for ti in range(nt):
                t = tc0 + ti
                out_ps = ps_out.tile([O, 128], F32, tag="ops")
                for k in range(K):
                    T = t * K + k
                    g = gt[:, ti * K + k, :]
                    s1 = vpool.tile([128, C], BF16, tag="s1")
                    nc.vector.scalar_tensor_tensor(s1[:], g[:, C:2 * C],
                                                   dyS[:, T:T + 1], g[:, 0:C],
                                                   op0=A.mult, op1=A.add)
                    s2 = vpool.tile([128, C], BF16, tag="s2")
                    nc.vector.scalar_tensor_tensor(s2[:], g[:, 3 * C:4 * C],
                                                   dyS[:, T:T + 1], g[:, 2 * C:3 * C],
                                                   op0=A.mult, op1=A.add)
                    v = vpool.tile([128, C], BF16, tag="v")
                    nc.vector.scalar_tensor_tensor(v[:], s2[:], dxS[:, T:T + 1], s1[:],
                                                   op0=A.mult, op1=A.add)
                    vT_ps = ps_tp.tile([C, 128], BF16, tag="vT")
                    nc.tensor.transpose(vT_ps[:], v[:], identb[:])
                    vT = vpool.tile([C, 128], BF16, tag="vTs")
                    nc.scalar.copy(vT[:], vT_ps[:])
                    nc.tensor.matmul(out_ps[:], wmat[:, k * O:(k + 1) * O], vT[:],
                                     start=(k == 0), stop=(k == K - 1))
                ot = opool.tile([O, 128], F32, tag="ot")
                nc.vector.tensor_scalar_add(ot[:], out_ps[:], bias[:])
                s = opool.tile([O, 1], F32, tag="sc")
                nc.vector.tensor_reduce(s[:], ot[:], axis=mybir.AxisListType.X,
                                        op=A.max, apply_absolute_value=True)
                nc.vector.tensor_scalar_max(s[:], s[:], 1e-30)
                rs = opool.tile([O, 1], F32, tag="rs")
                nc.vector.reciprocal(rs[:], s[:])
                qt = opool.tile([O, 128], I8, tag="qt")
                nc.vector.tensor_scalar(qt[:], ot[:], rs[:], 127.0,
                                        op0=A.mult, op1=A.mult)
                nc.sync.dma_start(y_out[:, t * 128:(t + 1) * 128], qt[:])
                nc.vector.tensor_scalar_mul(ysc_sb[:, t:t + 1], s[:], 1.0 / 127.0)
        nc.sync.dma_start(ysc_out, ysc_sb[:])


# ================= host side =================

def _prep_inputs(x, w_off, b_off, w, b):
    # [C, K*18]: col k*18+e = w_off[e, c, k]
    wofft = np.ascontiguousarray(
        w_off.reshape(18, C, K).transpose(1, 2, 0).reshape(C, K * 18)).astype(np.float32)
    wt = np.ascontiguousarray(
        w.reshape(O, C, K).transpose(1, 2, 0).reshape(C, K * O)).astype(ml_dtypes.bfloat16)
    p = np.arange(HWi)
    py, px = p // W, p % W
    kh = np.arange(K) // 3 - 1
    kw = np.arange(K) % 3 - 1
    # pypx[pp, t] = py of pixel t*128+pp (+PAD); cols NT.. hold px
    pypx = np.empty((128, 2 * NT), np.float32)
    pypx[:, :NT] = (py + PAD).reshape(NT, 128).T
    pypx[:, NT:] = (px + PAD).reshape(NT, 128).T
    kb = np.zeros((18,), np.float32)
    kb[0::2] = kh
    kb[1::2] = kw
    kb += b_off.reshape(18)
    kbB = np.ascontiguousarray(np.broadcast_to(kb, (128, 18)))
    bcol = np.ascontiguousarray(b.reshape(O, 1)).astype(np.float32)
    pack = (wt.tobytes() + wofft.tobytes() + pypx.tobytes()
            + kbB.tobytes() + bcol.tobytes())
    assert len(pack) == S_PACK
    shards = np.frombuffer(pack, np.uint8).reshape(N, 1, S8)
    xb = x.reshape(N, C, HWi).astype(ml_dtypes.bfloat16)
    return [{"x": xb[n], "cpack": shards[n]} for n in range(x.shape[0])]


_CACHED = {}


def _get_nc(num_devices=N, debug=False):
    key = (num_devices, debug)
    if key not in _CACHED:
        _CACHED[key] = build_kernel(num_devices=num_devices, debug=debug)
    return _CACHED[key]


def kernel(x, w_off, b_off, w, b):
    x = np.asarray(x, np.float32)
    nc = _get_nc()
    core_ins = _prep_inputs(x, np.asarray(w_off, np.float32),
                            np.asarray(b_off, np.float32),
                            np.asarray(w, np.float32), np.asarray(b, np.float32))
    res = bass_utils.run_bass_kernel_spmd(nc, core_ins, core_ids=list(range(N)))
    out = np.empty((N, O, H, W), np.float32)
    for n in range(N):
        q = res.results[n]["y"].reshape(O, NT, 128).astype(np.float32)
        sc = res.results[n]["ysc"].reshape(O, NT, 1).astype(np.float32)
        out[n] = (q * sc).reshape(O, H, W)
    return out



# revision 36
# speedup vs baseline: 1.4301x; 1.1978x over previous
"""DeformConvBlock Trainium2 kernel (data-parallel over batch across 8 cores).

Per-core (1 image, C=128, O=128, H=W=80, 3x3):
  1. offset = conv3x3(x, w_off) + b_off            (PE fp32 im2col GEMM)
  2. bilinear deform sampling via affine-basis identity:
       sample = P0[q] + dy*P1[q] + dx*P2[q] + dy*dx*P3[q],
     q = (floor(py), floor(px)) in an 8-padded image; P0..P3 = x and its
     v/h/cross shifted differences. One 1KB gather row per (tap,pixel).
  3. dma_gather 57.6K rows from DRAM [q, 4*C] bf16 -> (tap,pixel) rows on
     partitions; 3 scalar_tensor_tensor ops with per-partition dy/dx give
     the deformed im2col tile v[(k,p), c].
  4. PE transpose v tiles -> [c, p]; bf16 GEMM with w; + bias -> out.
"""

import contextlib
import os
BISECT = os.environ.get('KBISECT', '')
import numpy as np
import ml_dtypes

import jax
# Persistent compilation cache: run_bass_kernel_spmd builds a fresh jax.jit
# closure per call, so without this every call pays a full XLA recompile.
try:
    jax.config.update("jax_compilation_cache_dir", "/tmp/jax_comp_cache")
    jax.config.update("jax_persistent_cache_min_compile_time_secs", 0.0)
    jax.config.update("jax_persistent_cache_min_entry_size_bytes", -1)
except Exception:
    pass

import concourse.bass as bass
import concourse.tile as tile
from concourse import bacc, mybir
from concourse import bass_utils

F32 = mybir.dt.float32
BF16 = mybir.dt.bfloat16
I8 = mybir.dt.int8
I16 = mybir.dt.int16
I32 = mybir.dt.int32
A = mybir.AluOpType

N, C, O, H, W = 8, 128, 128, 80, 80
K = 9
# packed replicated constants (uploaded 1/N per core, AllGathered on device)
NB_WT = C * K * O * 2        # w_t   bf16 [C, K*O]
NB_WOFF = C * K * 18 * 4     # woff  f32  [C, K*18]
NB_PYPX = 128 * 100 * 4      # pypx  f32  [128, 2*NT]
NB_KB = 128 * 18 * 4         # kb    f32  [128, 18]
NB_B = 128 * 4               # b     f32  [O, 1]
S_PACK = NB_WT + NB_WOFF + NB_PYPX + NB_KB + NB_B
S8 = S_PACK // N
PAD = 8
WP = H + 2 * PAD          # 96
QP = WP * WP              # 9216
HWi = H * W               # 6400
NT = HWi // 128           # 50 pixel tiles
NTT = NT * K              # 450 gather tiles
NJ = NTT * 128            # 57600 gather rows
CLAMP_MAX = float(WP - 2)


def build_kernel(num_devices=N, debug=False):
    nc = bacc.Bacc("TRN2", target_bir_lowering=False, debug=False,
                   num_devices=num_devices)

    assert num_devices == N
    x_in = nc.dram_tensor("x", [C, HWi], BF16, kind="ExternalInput").ap()
    cpack_in = nc.dram_tensor("cpack", [1, S8], mybir.dt.uint8,
                              kind="ExternalInput").ap()

    # int8 output + per-(row, pixel-tile) scale appended as f32 bytes:
    # cols [0,HWi) = yq int8; cols [HWi, HWi+4*NT) = ysc f32, y = yq * ysc
    y_out = nc.dram_tensor("y", [O, HWi + 4 * NT], I8, kind="ExternalOutput").ap()
    dbg = {}
    if debug:
        for nm, shp, dt in (("off", [18, HWi], F32), ("idx", [C, NTT], I16),
                            ("dy", [C, NTT], F32), ("dx", [C, NTT], F32),
                            ("idxw", [C, NJ // 16], I16),
                            ("p4", [QP, 4 * C], BF16)):
            dbg[nm] = nc.dram_tensor("d_" + nm, shp, dt, kind="ExternalOutput").ap()

    p4_dram = nc.dram_tensor("p4_dram", [QP, 4 * C], BF16, kind="Internal").ap()
    idx_dram = nc.dram_tensor("idx_dram", [C, NTT], I16, kind="Internal").ap()

    with tile.TileContext(nc) as tc:
        with contextlib.ExitStack() as ctx:
            _body(ctx, tc, nc, x_in, cpack_in, y_out,
                  p4_dram, idx_dram, dbg)
    nc.compile()
    return nc


def _body(ctx, tc, nc, x_in, cpack_in, y_out,
          p4_dram, idx_dram, dbg):
    const = ctx.enter_context(tc.tile_pool(name="const", bufs=1))
    pers = ctx.enter_context(tc.tile_pool(name="pers", bufs=1))

    # ---- constants: AllGather the packed shard, then unpack ----
    ccd = ctx.enter_context(tc.tile_pool(name="ccd", bufs=1, space="DRAM"))
    cin = ccd.tile([1, S8], mybir.dt.uint8)
    cout = ccd.tile([1, S_PACK], mybir.dt.uint8)
    nc.gpsimd.dma_start(cin[:], cpack_in)
    nc.gpsimd.collective_compute(
        "AllGather", A.bypass, replica_groups=[list(range(N))],
        ins=[cin.opt()], outs=[cout.opt()])

    def unpack(off, nbytes, dt, parts):
        return cout[0, off:off + nbytes].bitcast(dt).rearrange(
            "(c f) -> c f", c=parts)

    iid = const.tile([128, 128], I32)
    nc.gpsimd.iota(iid[:], pattern=[[-1, 128]], base=0, channel_multiplier=1)
    ident = const.tile([128, 128], F32)
    nc.vector.tensor_scalar(ident[:], iid[:], 0, None, op0=A.is_equal)
    identb = const.tile([128, 128], BF16)
    nc.scalar.copy(identb[:], ident[:])
    o_wt, o_woff = 0, NB_WT
    o_pypx = o_woff + NB_WOFF
    o_kb = o_pypx + NB_PYPX
    o_b = o_kb + NB_KB
    wmat = const.tile([C, K * O], BF16)
    nc.sync.dma_start(wmat[:], unpack(o_wt, NB_WT, BF16, C))
    woff = const.tile([C, K * 18], F32)
    nc.sync.dma_start(woff[:], unpack(o_woff, NB_WOFF, F32, C))
    pypx = const.tile([128, 2 * NT], F32)
    nc.sync.dma_start(pypx[:], unpack(o_pypx, NB_PYPX, F32, 128))
    kbB = const.tile([128, 18], F32)
    nc.sync.dma_start(kbB[:], unpack(o_kb, NB_KB, F32, 128))
    bias = const.tile([O, 1], F32)
    nc.sync.dma_start(bias[:], unpack(o_b, NB_B, F32, O))

    # ---- persistent SBUF ----
    off_sb = pers.tile([18, HWi], F32)
    ysc_sb = pers.tile([O, NT], F32)
    idxS = pers.tile([C, NTT], I16)
    dyS = pers.tile([C, NTT], F32)
    dxS = pers.tile([C, NTT], F32)
    idxW = pers.tile([C, NJ // 16], I16)

    # ================= phase 1: load, offset conv, planes, P4 =================
    with tc.tile_pool(name="ph1", bufs=1) as ph1, \
         tc.tile_pool(name="ph1s", bufs=3) as ph1s, \
         tc.tile_pool(name="ps_off", bufs=2, space="PSUM") as ps_off, \
         tc.tile_pool(name="ps_tp1", bufs=3, space="PSUM") as ps_tp:
        xp = ph1.tile([C, QP], BF16)
        nc.gpsimd.memset(xp[:], 0.0)
        xp3 = xp[:].rearrange("c (h w) -> c h w", h=WP)
        nc.sync.dma_start(xp3[:, PAD:PAD + H, PAD:PAD + W],
                          x_in.rearrange("c (h w) -> c h w", h=H))
        # fp32 upcast for the offset conv (keeps offset precision)
        xf = ph1.tile([C, QP], F32)
        nc.scalar.copy(xf[:], xp[:])
        xf3 = xf[:].rearrange("c (h w) -> c h w", h=WP)

        # offset conv (fp32), chunks of 6 output rows (N=480)
        CH = 6
        for yc in range(0, H, CH):
            rows = min(CH, H - yc)
            po = ps_off.tile([18, CH * W], F32, tag="po")
            for k in range(K):
                kh, kw = divmod(k, 3)
                rhs = xf3[:, (yc + kh - 1 + PAD):(yc + kh - 1 + PAD) + rows,
                          (kw - 1 + PAD):(kw - 1 + PAD) + W]
                nc.tensor.matmul(po[:, :rows * W],
                                 woff[:, k * 18:(k + 1) * 18], rhs,
                                 start=(k == 0), stop=(k == K - 1))
            nc.scalar.copy(off_sb[:, yc * W:(yc + rows) * W], po[:, :rows * W])
        if dbg:
            nc.sync.dma_start(dbg["off"], off_sb[:])

        # bf16 planes
        xb = xp
        d1 = ph1.tile([C, QP], BF16)
        nc.gpsimd.memset(d1[:, QP - WP:], 0.0)
        nc.vector.tensor_tensor(d1[:, :QP - WP], xb[:, WP:], xb[:, :QP - WP], op=A.subtract)
        d2 = ph1.tile([C, QP], BF16)
        nc.gpsimd.memset(d2[:, QP - 1:], 0.0)
        nc.vector.tensor_tensor(d2[:, :QP - 1], xb[:, 1:], xb[:, :QP - 1], op=A.subtract)
        d3 = ph1.tile([C, QP], BF16)
        nc.gpsimd.memset(d3[:, QP - WP:], 0.0)
        nc.vector.tensor_tensor(d3[:, :QP - WP], d2[:, WP:], d2[:, :QP - WP], op=A.subtract)
        planes = [xb, d1, d2, d3]

        # zero all of P4 first (pads), then overwrite the active interior
        zbuf = ph1.tile([128, 8192], BF16)
        nc.gpsimd.memset(zbuf[:], 0.0)
        ZR = 2048  # rows per zero DMA (keeps every AP dim < 2^16)
        for r0 in range(0, QP, ZR):
            rows = min(ZR, QP - r0)
            nc.sync.dma_start(p4_dram[r0:r0 + rows, :],
                              zbuf[:, 0:rows * 512 // 128])

        # active region: rows/cols [PAD-1, PAD+H) of the padded image
        WA = W + 1  # 81
        for y in range(-1, H):
            qp0 = (y + PAD) * WP + (PAD - 1)
            stg = ph1s.tile([WA, 4 * C], BF16, tag="stg")
            for pi, pl in enumerate(planes):
                tp = ps_tp.tile([WA, 128], BF16, tag="tpp")
                nc.tensor.transpose(tp[:], pl[:, qp0:qp0 + WA], identb[:])
                nc.scalar.copy(stg[:, pi * C:(pi + 1) * C], tp[:])
            nc.sync.dma_start(p4_dram[qp0:qp0 + WA, :], stg[:])

    if dbg:
        nc.sync.dma_start(dbg["p4"], p4_dram)

    # ================= phase 2: maps =================
    with tc.tile_pool(name="ph2s", bufs=3) as sm, \
         tc.tile_pool(name="ps_tp2", bufs=2, space="PSUM") as ps_tp:
        for t in range(NT):
            offT_ps = ps_tp.tile([128, 18], F32, tag="offT")
            nc.tensor.transpose(offT_ps[:], off_sb[:, t * 128:(t + 1) * 128],
                                ident[0:18, 0:18])
            q = sm.tile([128, 18], F32, tag="mq")
            nc.scalar.copy(q[:], offT_ps[:])
            nc.vector.tensor_tensor(q[:], q[:], kbB[:], op=A.add)
            q2v = q[:].rearrange("p (k two) -> p k two", two=2)
            nc.vector.tensor_scalar_add(q2v[:, :, 0], q2v[:, :, 0],
                                        pypx[:, t:t + 1])
            nc.vector.tensor_scalar_add(q2v[:, :, 1], q2v[:, :, 1],
                                        pypx[:, NT + t:NT + t + 1])
            nc.vector.tensor_scalar_min(q[:], q[:], CLAMP_MAX)
            nc.vector.tensor_scalar_max(q[:], q[:], 0.0)
            qi = sm.tile([128, 18], I32, tag="mqi")
            nc.vector.tensor_copy(qi[:], q[:])          # rne
            qr = sm.tile([128, 18], F32, tag="mqr")
            nc.vector.tensor_copy(qr[:], qi[:])
            m = sm.tile([128, 18], F32, tag="mm")
            nc.vector.tensor_tensor(m[:], qr[:], q[:], op=A.is_gt)
            fl = sm.tile([128, 18], F32, tag="mfl")
            nc.vector.tensor_tensor(fl[:], qr[:], m[:], op=A.subtract)
            dd = sm.tile([128, 18], F32, tag="mdd")
            nc.vector.tensor_tensor(dd[:], q[:], fl[:], op=A.subtract)
            fl2 = fl[:].rearrange("p (k two) -> p k two", two=2)
            dd2 = dd[:].rearrange("p (k two) -> p k two", two=2)
            fidx = sm.tile([128, 9], F32, tag="mfi")
            nc.vector.scalar_tensor_tensor(fidx[:], fl2[:, :, 0], float(WP),
                                           fl2[:, :, 1], op0=A.mult, op1=A.add)
            nc.vector.tensor_copy(idxS[:, t * K:(t + 1) * K], fidx[:])
            nc.vector.tensor_copy(dyS[:, t * K:(t + 1) * K], dd2[:, :, 0])
            nc.vector.tensor_copy(dxS[:, t * K:(t + 1) * K], dd2[:, :, 1])

        # idx wrap: j = T*128+pp -> wrapped[pp%16, 8T + pp//16]
        if BISECT == 'B':
            nc.gpsimd.memset(idxW[:], 0)
        else:
            nc.sync.dma_start(idx_dram, idxS[:])
            w1 = sm.tile([16, 8 * NTT], I16, tag="w1")
            src2 = idx_dram.rearrange("(u r) t -> r u t", u=8)
            nc.sync.dma_start(w1[:].rearrange("r (u t) -> r u t", u=8), src2)
            w1v = w1[:].rearrange("r (u t) -> r t u", u=8)
            nc.vector.tensor_copy(idxW[0:16, :].rearrange("r (t u) -> r t u", u=8), w1v)
            for g in range(1, 8):
                nc.sync.dma_start(idxW[16 * g:16 * (g + 1), :], idxW[0:16, :])
    if dbg:
        nc.sync.dma_start(dbg["idx"], idxS[:])
        nc.sync.dma_start(dbg["dy"], dyS[:])
        nc.sync.dma_start(dbg["dx"], dxS[:])
        nc.sync.dma_start(dbg["idxw"], idxW[:])

    # ================= phase 3: gather + combine + GEMM =================
    if BISECT == 'A':
        with tc.tile_pool(name="za", bufs=1) as za:
            zy = za.tile([O, 400], I8)
            nc.gpsimd.memset(zy[:], 0)
            for t0 in range(0, HWi, 400):
                nc.sync.dma_start(y_out[:, t0:t0 + 400], zy[:])
            nc.sync.dma_start(y_out[:, HWi:HWi + 4 * NT], zy[:, :4 * NT])
        return
    TCH = 2
    with tc.tile_pool(name="gpool", bufs=3) as gpool, \
         tc.tile_pool(name="vpool", bufs=4) as vpool, \
         tc.tile_pool(name="opool", bufs=3) as opool, \
         tc.tile_pool(name="ps_out", bufs=2, space="PSUM") as ps_out, \
         tc.tile_pool(name="ps_tp3", bufs=3, space="PSUM") as ps_tp:
        for tc0 in range(0, NT, TCH):
            nt = min(TCH, NT - tc0)
            nidx = nt * K * 128
            gt = gpool.tile([128, TCH * K, 4 * C], BF16, tag="gather")
            c0 = tc0 * K * 8
            nc.gpsimd.dma_gather(gt[:, :nt * K, :], p4_dram,
                                 idxW[:, c0:c0 + nidx // 16],
                                 num_idxs=nidx, num_idxs_reg=nidx, elem_size=4 * C,
                                 single_packet=False)
            for ti in range(nt):
                t = tc0 + ti
                out_ps = ps_out.tile([O, 128], F32, tag="ops")
                for k in range(K):
                    T = t * K + k
                    g = gt[:, ti * K + k, :]
                    s1 = vpool.tile([128, C], BF16, tag="s1")
                    nc.vector.scalar_tensor_tensor(s1[:], g[:, C:2 * C],
                                                   dyS[:, T:T + 1], g[:, 0:C],
                                                   op0=A.mult, op1=A.add)
                    s2 = vpool.tile([128, C], BF16, tag="s2")
                    nc.vector.scalar_tensor_tensor(s2[:], g[:, 3 * C:4 * C],
                                                   dyS[:, T:T + 1], g[:, 2 * C:3 * C],
                                                   op0=A.mult, op1=A.add)
                    v = vpool.tile([128, C], BF16, tag="v")
                    nc.vector.scalar_tensor_tensor(v[:], s2[:], dxS[:, T:T + 1], s1[:],
                                                   op0=A.mult, op1=A.add)
                    vT_ps = ps_tp.tile([C, 128], BF16, tag="vT")
                    nc.tensor.transpose(vT_ps[:], v[:], identb[:])
                    vT = vpool.tile([C, 128], BF16, tag="vTs")
                    nc.scalar.copy(vT[:], vT_ps[:])
                    nc.tensor.matmul(out_ps[:], wmat[:, k * O:(k + 1) * O], vT[:],
                                     start=(k == 0), stop=(k == K - 1))
                ot = opool.tile([O, 128], F32, tag="ot")
                nc.vector.tensor_scalar_add(ot[:], out_ps[:], bias[:])
                s = opool.tile([O, 1], F32, tag="sc")
                nc.vector.tensor_reduce(s[:], ot[:], axis=mybir.AxisListType.X,
                                        op=A.max, apply_absolute_value=True)
                nc.vector.tensor_scalar_max(s[:], s[:], 1e-30)
                rs = opool.tile([O, 1], F32, tag="rs")
                nc.vector.reciprocal(rs[:], s[:])
                qt = opool.tile([O, 128], I8, tag="qt")
                nc.vector.tensor_scalar(qt[:], ot[:], rs[:], 127.0,
                                        op0=A.mult, op1=A.mult)
                nc.sync.dma_start(y_out[:, t * 128:(t + 1) * 128], qt[:])
                nc.vector.tensor_scalar_mul(ysc_sb[:, t:t + 1], s[:], 1.0 / 127.0)
        nc.sync.dma_start(y_out[:, HWi:HWi + 4 * NT].bitcast(F32), ysc_sb[:])


# ================= host side =================

def _prep_inputs(x, w_off, b_off, w, b):
    # [C, K*18]: col k*18+e = w_off[e, c, k]
    wofft = np.ascontiguousarray(
        w_off.reshape(18, C, K).transpose(1, 2, 0).reshape(C, K * 18)).astype(np.float32)
    wt = np.ascontiguousarray(
        w.reshape(O, C, K).transpose(1, 2, 0).reshape(C, K * O)).astype(ml_dtypes.bfloat16)
    p = np.arange(HWi)
    py, px = p // W, p % W
    kh = np.arange(K) // 3 - 1
    kw = np.arange(K) % 3 - 1
    # pypx[pp, t] = py of pixel t*128+pp (+PAD); cols NT.. hold px
    pypx = np.empty((128, 2 * NT), np.float32)
    pypx[:, :NT] = (py + PAD).reshape(NT, 128).T
    pypx[:, NT:] = (px + PAD).reshape(NT, 128).T
    kb = np.zeros((18,), np.float32)
    kb[0::2] = kh
    kb[1::2] = kw
    kb += b_off.reshape(18)
    kbB = np.ascontiguousarray(np.broadcast_to(kb, (128, 18)))
    bcol = np.ascontiguousarray(b.reshape(O, 1)).astype(np.float32)
    pack = (wt.tobytes() + wofft.tobytes() + pypx.tobytes()
            + kbB.tobytes() + bcol.tobytes())
    assert len(pack) == S_PACK
    shards = np.frombuffer(pack, np.uint8).reshape(N, 1, S8)
    xb = x.reshape(N, C, HWi).astype(ml_dtypes.bfloat16)
    return [{"x": xb[n], "cpack": shards[n]} for n in range(x.shape[0])]


_CACHED = {}


def _get_nc(num_devices=N, debug=False):
    key = (num_devices, debug)
    if key not in _CACHED:
        _CACHED[key] = build_kernel(num_devices=num_devices, debug=debug)
    return _CACHED[key]


def kernel(x, w_off, b_off, w, b):
    x = np.asarray(x, np.float32)
    nc = _get_nc()
    core_ins = _prep_inputs(x, np.asarray(w_off, np.float32),
                            np.asarray(b_off, np.float32),
                            np.asarray(w, np.float32), np.asarray(b, np.float32))
    res = bass_utils.run_bass_kernel_spmd(nc, core_ins, core_ids=list(range(N)))
    out = np.empty((N, O, H, W), np.float32)
    for n in range(N):
        yr = res.results[n]["y"]
        q = yr[:, :HWi].reshape(O, NT, 128).astype(np.float32)
        sc = np.ascontiguousarray(yr[:, HWi:]).view(np.float32).reshape(O, NT, 1)
        out[n] = (q * sc).reshape(O, H, W)
    return out



# revision 47
# speedup vs baseline: 1.7452x; 1.2203x over previous
"""DeformConvBlock Trainium2 kernel (data-parallel over batch across 8 cores).

Per-core (1 image, C=128, O=128, H=W=80, 3x3):
  1. offset = conv3x3(x, w_off) + b_off            (PE fp32 im2col GEMM)
  2. bilinear deform sampling via affine-basis identity:
       sample = P0[q] + dy*P1[q] + dx*P2[q] + dy*dx*P3[q],
     q = (floor(py), floor(px)) in an 8-padded image; P0..P3 = x and its
     v/h/cross shifted differences. One 1KB gather row per (tap,pixel).
  3. dma_gather 57.6K rows from DRAM [q, 4*C] bf16 -> (tap,pixel) rows on
     partitions; 3 scalar_tensor_tensor ops with per-partition dy/dx give
     the deformed im2col tile v[(k,p), c].
  4. PE transpose v tiles -> [c, p]; bf16 GEMM with w; + bias -> out.
"""

import contextlib
import os
BISECT = os.environ.get('KBISECT', '')
SIM_BUILD = bool(os.environ.get('KSIM'))  # collective-free single-core build
                                          # for TimelineSim; never set by the
                                          # harness
import numpy as np
import ml_dtypes

import jax
# Persistent compilation cache: run_bass_kernel_spmd builds a fresh jax.jit
# closure per call, so without this every call pays a full XLA recompile.
try:
    jax.config.update("jax_compilation_cache_dir", "/tmp/jax_comp_cache")
    jax.config.update("jax_persistent_cache_min_compile_time_secs", 0.0)
    jax.config.update("jax_persistent_cache_min_entry_size_bytes", -1)
except Exception:
    pass

import concourse.bass as bass
import concourse.tile as tile
from concourse import bacc, mybir
from concourse import bass_utils

F32 = mybir.dt.float32
BF16 = mybir.dt.bfloat16
I8 = mybir.dt.int8
I16 = mybir.dt.int16
I32 = mybir.dt.int32
A = mybir.AluOpType

N, C, O, H, W = 8, 128, 128, 80, 80
K = 9
# packed replicated constants (uploaded 1/N per core as trailing columns of
# the x tensor, AllGathered on device)
NB_WT = C * K * O * 2        # w_t   bf16 [C, K*O]
NB_WOFF = C * K * 18 * 4     # woff  f32  [C, K*18]
NB_PYPX = 128 * 100 * 4      # pypx  f32  [128, 2*NT]
NB_KB = 128 * 18 * 4         # kb    f32  [128, 18]
NB_B = 128 * 4               # b     f32  [O, 1]
S_RAW = NB_WT + NB_WOFF + NB_PYPX + NB_KB + NB_B
S8 = -(-S_RAW // (N * 256)) * 256    # per-core shard, 256B-aligned
S_PACK = S8 * N
XCOLS = S8 // 256                    # shard as [128, XCOLS] bf16 cols of x
PAD = 8
WP = H + 2 * PAD          # 96
QP = WP * WP              # 9216
HWi = H * W               # 6400
NT = HWi // 128           # 50 pixel tiles
NTT = NT * K              # 450 gather tiles
NJ = NTT * 128            # 57600 gather rows
CLAMP_MAX = float(WP - 2)


def build_kernel(num_devices=N, debug=False):
    nc = bacc.Bacc("TRN2", target_bir_lowering=False, debug=False,
                   num_devices=num_devices)

    assert num_devices == N or SIM_BUILD
    # cols [0,HWi) = image; cols [HWi, HWi+XCOLS) = this core's constant shard
    x_in = nc.dram_tensor("x", [C, HWi + XCOLS], BF16, kind="ExternalInput").ap()

    # int8 output + per-(row, pixel-tile) scale appended as f32 bytes:
    # cols [0,HWi) = yq int8; cols [HWi, HWi+4*NT) = ysc f32, y = yq * ysc
    y_out = nc.dram_tensor("y", [O, HWi + 4 * NT], I8, kind="ExternalOutput").ap()
    dbg = {}
    if debug:
        for nm, shp, dt in (("off", [18, HWi], F32), ("idx", [C, NTT], I16),
                            ("dy", [C, NTT], F32), ("dx", [C, NTT], F32),
                            ("idxw", [C, NJ // 16], I16),
                            ("p4", [QP, 4 * C], BF16)):
            dbg[nm] = nc.dram_tensor("d_" + nm, shp, dt, kind="ExternalOutput").ap()

    p4_dram = nc.dram_tensor("p4_dram", [QP, 4 * C], BF16, kind="Internal").ap()
    idx_dram = nc.dram_tensor("idx_dram", [C, NTT], I16, kind="Internal").ap()

    with tile.TileContext(nc) as tc:
        with contextlib.ExitStack() as ctx:
            _body(ctx, tc, nc, x_in, y_out, p4_dram, idx_dram, dbg)
    nc.compile()
    return nc


def _body(ctx, tc, nc, x_in, y_out, p4_dram, idx_dram, dbg):
    const = ctx.enter_context(tc.tile_pool(name="const", bufs=1))
    pers = ctx.enter_context(tc.tile_pool(name="pers", bufs=1))

    # ---- constants: AllGather the packed shard, then unpack ----
    ccd = ctx.enter_context(tc.tile_pool(name="ccd", bufs=1, space="DRAM"))
    cin = ccd.tile([1, S8], mybir.dt.uint8)
    cout = ccd.tile([1, S_PACK], mybir.dt.uint8)
    cin_v = cin[0, :].bitcast(BF16).rearrange("(c f) -> c f", c=128)
    nc.gpsimd.dma_start(cin_v, x_in[:, HWi:HWi + XCOLS])
    if SIM_BUILD:
        for r in range(N):
            nc.gpsimd.dma_start(cout[0, r * S8:(r + 1) * S8], cin[0, :])
    else:
        nc.gpsimd.collective_compute(
            "AllGather", A.bypass, replica_groups=[list(range(N))],
            ins=[cin.opt()], outs=[cout.opt()])

    def unpack(off, nbytes, dt, parts):
        return cout[0, off:off + nbytes].bitcast(dt).rearrange(
            "(c f) -> c f", c=parts)

    iid = const.tile([128, 128], I32)
    nc.gpsimd.iota(iid[:], pattern=[[-1, 128]], base=0, channel_multiplier=1)
    ident = const.tile([128, 128], F32)
    nc.vector.tensor_scalar(ident[:], iid[:], 0, None, op0=A.is_equal)
    identb = const.tile([128, 128], BF16)
    nc.scalar.copy(identb[:], ident[:])
    o_wt, o_woff = 0, NB_WT
    o_pypx = o_woff + NB_WOFF
    o_kb = o_pypx + NB_PYPX
    o_b = o_kb + NB_KB
    wmat = const.tile([C, K * O], BF16)
    nc.sync.dma_start(wmat[:], unpack(o_wt, NB_WT, BF16, C))
    woff = const.tile([C, K * 18], F32)
    nc.sync.dma_start(woff[:], unpack(o_woff, NB_WOFF, F32, C))
    pypx = const.tile([128, 2 * NT], F32)
    nc.sync.dma_start(pypx[:], unpack(o_pypx, NB_PYPX, F32, 128))
    kbB = const.tile([128, 18], F32)
    nc.sync.dma_start(kbB[:], unpack(o_kb, NB_KB, F32, 128))
    bias = const.tile([O, 1], F32)
    nc.sync.dma_start(bias[:], unpack(o_b, NB_B, F32, O))

    # ---- persistent SBUF ----
    off_sb = pers.tile([18, HWi], F32)
    ysc_sb = pers.tile([O, NT], F32)
    idxS = pers.tile([C, NTT], I16)
    dyS = pers.tile([C, NTT], F32)
    dxS = pers.tile([C, NTT], F32)
    idxW = pers.tile([C, NJ // 16], I16)

    # ================= phase 1: load, offset conv, planes, P4 =================
    with tc.tile_pool(name="ph1", bufs=1) as ph1, \
         tc.tile_pool(name="ph1s", bufs=3) as ph1s, \
         tc.tile_pool(name="ps_off", bufs=2, space="PSUM") as ps_off, \
         tc.tile_pool(name="ps_tp1", bufs=3, space="PSUM") as ps_tp:
        xp = ph1.tile([C, QP], BF16)
        nc.gpsimd.memset(xp[:], 0.0)
        xp3 = xp[:].rearrange("c (h w) -> c h w", h=WP)
        nc.sync.dma_start(xp3[:, PAD:PAD + H, PAD:PAD + W],
                          x_in[:, :HWi].rearrange("c (h w) -> c h w", h=H))
        # fp32 upcast for the offset conv (keeps offset precision)
        xf = ph1.tile([C, QP], F32)
        nc.scalar.copy(xf[:], xp[:])
        xf3 = xf[:].rearrange("c (h w) -> c h w", h=WP)

        # offset conv (fp32), chunks of 6 output rows (N=480)
        CH = 6
        for yc in range(0, H, CH):
            rows = min(CH, H - yc)
            po = ps_off.tile([18, CH * W], F32, tag="po")
            for k in range(K):
                kh, kw = divmod(k, 3)
                rhs = xf3[:, (yc + kh - 1 + PAD):(yc + kh - 1 + PAD) + rows,
                          (kw - 1 + PAD):(kw - 1 + PAD) + W]
                nc.tensor.matmul(po[:, :rows * W],
                                 woff[:, k * 18:(k + 1) * 18], rhs,
                                 start=(k == 0), stop=(k == K - 1))
            nc.scalar.copy(off_sb[:, yc * W:(yc + rows) * W], po[:, :rows * W])
        if dbg:
            nc.sync.dma_start(dbg["off"], off_sb[:])

        # bf16 planes
        xb = xp
        d1 = ph1.tile([C, QP], BF16)
        nc.gpsimd.memset(d1[:, QP - WP:], 0.0)
        nc.vector.tensor_tensor(d1[:, :QP - WP], xb[:, WP:], xb[:, :QP - WP], op=A.subtract)
        d2 = ph1.tile([C, QP], BF16)
        nc.gpsimd.memset(d2[:, QP - 1:], 0.0)
        nc.vector.tensor_tensor(d2[:, :QP - 1], xb[:, 1:], xb[:, :QP - 1], op=A.subtract)
        d3 = ph1.tile([C, QP], BF16)
        nc.gpsimd.memset(d3[:, QP - WP:], 0.0)
        nc.vector.tensor_tensor(d3[:, :QP - WP], d2[:, WP:], d2[:, :QP - WP], op=A.subtract)
        planes = [xb, d1, d2, d3]

        # zero all of P4 first (pads), then overwrite the active interior
        zbuf = ph1.tile([128, 8192], BF16)
        nc.gpsimd.memset(zbuf[:], 0.0)
        ZR = 2048  # rows per zero DMA (keeps every AP dim < 2^16)
        for r0 in range(0, QP, ZR):
            rows = min(ZR, QP - r0)
            nc.sync.dma_start(p4_dram[r0:r0 + rows, :],
                              zbuf[:, 0:rows * 512 // 128])

        # active region: rows/cols [PAD-1, PAD+H) of the padded image
        WA = W + 1  # 81
        for y in range(-1, H):
            qp0 = (y + PAD) * WP + (PAD - 1)
            stg = ph1s.tile([WA, 4 * C], BF16, tag="stg")
            for pi, pl in enumerate(planes):
                tp = ps_tp.tile([WA, 128], BF16, tag="tpp")
                nc.tensor.transpose(tp[:], pl[:, qp0:qp0 + WA], identb[:])
                nc.scalar.copy(stg[:, pi * C:(pi + 1) * C], tp[:])
            nc.sync.dma_start(p4_dram[qp0:qp0 + WA, :], stg[:])

    if dbg:
        nc.sync.dma_start(dbg["p4"], p4_dram)

    # ================= phase 2: maps =================
    with tc.tile_pool(name="ph2s", bufs=3) as sm, \
         tc.tile_pool(name="ps_tp2", bufs=2, space="PSUM") as ps_tp:
        for t in range(NT):
            offT_ps = ps_tp.tile([128, 18], F32, tag="offT")
            nc.tensor.transpose(offT_ps[:], off_sb[:, t * 128:(t + 1) * 128],
                                ident[0:18, 0:18])
            q = sm.tile([128, 18], F32, tag="mq")
            nc.scalar.copy(q[:], offT_ps[:])
            nc.vector.tensor_tensor(q[:], q[:], kbB[:], op=A.add)
            q2v = q[:].rearrange("p (k two) -> p k two", two=2)
            nc.vector.tensor_scalar_add(q2v[:, :, 0], q2v[:, :, 0],
                                        pypx[:, t:t + 1])
            nc.vector.tensor_scalar_add(q2v[:, :, 1], q2v[:, :, 1],
                                        pypx[:, NT + t:NT + t + 1])
            nc.vector.tensor_scalar_min(q[:], q[:], CLAMP_MAX)
            nc.vector.tensor_scalar_max(q[:], q[:], 0.0)
            qi = sm.tile([128, 18], I32, tag="mqi")
            nc.vector.tensor_copy(qi[:], q[:])          # rne
            qr = sm.tile([128, 18], F32, tag="mqr")
            nc.vector.tensor_copy(qr[:], qi[:])
            m = sm.tile([128, 18], F32, tag="mm")
            nc.vector.tensor_tensor(m[:], qr[:], q[:], op=A.is_gt)
            fl = sm.tile([128, 18], F32, tag="mfl")
            nc.vector.tensor_tensor(fl[:], qr[:], m[:], op=A.subtract)
            dd = sm.tile([128, 18], F32, tag="mdd")
            nc.vector.tensor_tensor(dd[:], q[:], fl[:], op=A.subtract)
            fl2 = fl[:].rearrange("p (k two) -> p k two", two=2)
            dd2 = dd[:].rearrange("p (k two) -> p k two", two=2)
            fidx = sm.tile([128, 9], F32, tag="mfi")
            nc.vector.scalar_tensor_tensor(fidx[:], fl2[:, :, 0], float(WP),
                                           fl2[:, :, 1], op0=A.mult, op1=A.add)
            nc.vector.tensor_copy(idxS[:, t * K:(t + 1) * K], fidx[:])
            nc.vector.tensor_copy(dyS[:, t * K:(t + 1) * K], dd2[:, :, 0])
            nc.vector.tensor_copy(dxS[:, t * K:(t + 1) * K], dd2[:, :, 1])

        # idx wrap: j = T*128+pp -> wrapped[pp%16, 8T + pp//16]
        if BISECT == 'B':
            nc.gpsimd.memset(idxW[:], 0)
        else:
            nc.sync.dma_start(idx_dram, idxS[:])
            w1 = sm.tile([16, 8 * NTT], I16, tag="w1")
            src2 = idx_dram.rearrange("(u r) t -> r u t", u=8)
            nc.sync.dma_start(w1[:].rearrange("r (u t) -> r u t", u=8), src2)
            w1v = w1[:].rearrange("r (u t) -> r t u", u=8)
            nc.vector.tensor_copy(idxW[0:16, :].rearrange("r (t u) -> r t u", u=8), w1v)
            for g in range(1, 8):
                nc.sync.dma_start(idxW[16 * g:16 * (g + 1), :], idxW[0:16, :])
    if dbg:
        nc.sync.dma_start(dbg["idx"], idxS[:])
        nc.sync.dma_start(dbg["dy"], dyS[:])
        nc.sync.dma_start(dbg["dx"], dxS[:])
        nc.sync.dma_start(dbg["idxw"], idxW[:])

    # ================= phase 3: gather + combine + GEMM =================
    if BISECT == 'A':
        with tc.tile_pool(name="za", bufs=1) as za:
            zy = za.tile([O, 400], I8)
            nc.gpsimd.memset(zy[:], 0)
            for t0 in range(0, HWi, 400):
                nc.sync.dma_start(y_out[:, t0:t0 + 400], zy[:])
            nc.sync.dma_start(y_out[:, HWi:HWi + 4 * NT], zy[:, :4 * NT])
        return
    TCH = 2
    DO_GATHER = BISECT != 'C'
    DO_COMPUTE = BISECT != 'G'
    if not DO_COMPUTE:
        nc.gpsimd.memset(ysc_sb[:], 0.0)
    with tc.tile_pool(name="gpool", bufs=3) as gpool, \
         tc.tile_pool(name="vpool", bufs=4) as vpool, \
         tc.tile_pool(name="opool", bufs=3) as opool, \
         tc.tile_pool(name="ps_out", bufs=2, space="PSUM") as ps_out, \
         tc.tile_pool(name="ps_tp3", bufs=3, space="PSUM") as ps_tp:
        for tc0 in range(0, NT, TCH):
            nt = min(TCH, NT - tc0)
            nidx = nt * K * 128
            gt = gpool.tile([128, TCH * K, 4 * C], BF16, tag="gather")
            c0 = tc0 * K * 8
            if DO_GATHER:
                nc.gpsimd.dma_gather(gt[:, :nt * K, :], p4_dram,
                                     idxW[:, c0:c0 + nidx // 16],
                                     num_idxs=nidx, num_idxs_reg=nidx, elem_size=4 * C,
                                     single_packet=False)
            else:
                nc.gpsimd.memset(gt[:, :nt * K, :], 0.0)
            if not DO_COMPUTE:
                continue
            for ti in range(nt):
                t = tc0 + ti
                out_ps = ps_out.tile([O, 128], F32, tag="ops")
                for k in range(K):
                    T = t * K + k
                    g = gt[:, ti * K + k, :]
                    s1 = vpool.tile([128, C], BF16, tag="s1")
                    nc.vector.scalar_tensor_tensor(s1[:], g[:, C:2 * C],
                                                   dyS[:, T:T + 1], g[:, 0:C],
                                                   op0=A.mult, op1=A.add)
                    s2 = vpool.tile([128, C], BF16, tag="s2")
                    nc.vector.scalar_tensor_tensor(s2[:], g[:, 3 * C:4 * C],
                                                   dyS[:, T:T + 1], g[:, 2 * C:3 * C],
                                                   op0=A.mult, op1=A.add)
                    v = vpool.tile([128, C], BF16, tag="v")
                    nc.vector.scalar_tensor_tensor(v[:], s2[:], dxS[:, T:T + 1], s1[:],
                                                   op0=A.mult, op1=A.add)
                    vT_ps = ps_tp.tile([C, 128], BF16, tag="vT")
                    nc.tensor.transpose(vT_ps[:], v[:], identb[:])
                    vT = vpool.tile([C, 128], BF16, tag="vTs")
                    nc.scalar.copy(vT[:], vT_ps[:])
                    nc.tensor.matmul(out_ps[:], wmat[:, k * O:(k + 1) * O], vT[:],
                                     start=(k == 0), stop=(k == K - 1))
                ot = opool.tile([O, 128], F32, tag="ot")
                nc.vector.tensor_scalar_add(ot[:], out_ps[:], bias[:])
                s = opool.tile([O, 1], F32, tag="sc")
                nc.vector.tensor_reduce(s[:], ot[:], axis=mybir.AxisListType.X,
                                        op=A.max, apply_absolute_value=True)
                nc.vector.tensor_scalar_max(s[:], s[:], 1e-30)
                rs = opool.tile([O, 1], F32, tag="rs")
                nc.vector.reciprocal(rs[:], s[:])
                qt = opool.tile([O, 128], I8, tag="qt")
                nc.vector.tensor_scalar(qt[:], ot[:], rs[:], 127.0,
                                        op0=A.mult, op1=A.mult)
                nc.sync.dma_start(y_out[:, t * 128:(t + 1) * 128], qt[:])
                nc.vector.tensor_scalar_mul(ysc_sb[:, t:t + 1], s[:], 1.0 / 127.0)
        nc.sync.dma_start(y_out[:, HWi:HWi + 4 * NT].bitcast(F32), ysc_sb[:])


# ================= host side =================

def _prep_inputs(x, w_off, b_off, w, b):
    # [C, K*18]: col k*18+e = w_off[e, c, k]
    wofft = np.ascontiguousarray(
        w_off.reshape(18, C, K).transpose(1, 2, 0).reshape(C, K * 18)).astype(np.float32)
    wt = np.ascontiguousarray(
        w.reshape(O, C, K).transpose(1, 2, 0).reshape(C, K * O)).astype(ml_dtypes.bfloat16)
    p = np.arange(HWi)
    py, px = p // W, p % W
    kh = np.arange(K) // 3 - 1
    kw = np.arange(K) % 3 - 1
    # pypx[pp, t] = py of pixel t*128+pp (+PAD); cols NT.. hold px
    pypx = np.empty((128, 2 * NT), np.float32)
    pypx[:, :NT] = (py + PAD).reshape(NT, 128).T
    pypx[:, NT:] = (px + PAD).reshape(NT, 128).T
    kb = np.zeros((18,), np.float32)
    kb[0::2] = kh
    kb[1::2] = kw
    kb += b_off.reshape(18)
    kbB = np.ascontiguousarray(np.broadcast_to(kb, (128, 18)))
    bcol = np.ascontiguousarray(b.reshape(O, 1)).astype(np.float32)
    pack = (wt.tobytes() + wofft.tobytes() + pypx.tobytes()
            + kbB.tobytes() + bcol.tobytes())
    pack += b"\x00" * (S_PACK - len(pack))
    shards = np.frombuffer(pack, np.uint8).reshape(N, S8)
    xb = x.reshape(N, C, HWi).astype(ml_dtypes.bfloat16)
    xcat = np.concatenate(
        [xb, shards.view(ml_dtypes.bfloat16).reshape(N, C, XCOLS)], axis=2)
    return [{"x": xcat[n]} for n in range(N)]


_CACHED = {}


def _get_nc(num_devices=N, debug=False):
    key = (num_devices, debug)
    if key not in _CACHED:
        _CACHED[key] = build_kernel(num_devices=num_devices, debug=debug)
    return _CACHED[key]


def kernel(x, w_off, b_off, w, b):
    x = np.asarray(x, np.float32)
    nc = _get_nc()
    core_ins = _prep_inputs(x, np.asarray(w_off, np.float32),
                            np.asarray(b_off, np.float32),
                            np.asarray(w, np.float32), np.asarray(b, np.float32))
    res = bass_utils.run_bass_kernel_spmd(nc, core_ins, core_ids=list(range(N)))
    out = np.empty((N, O, H, W), np.float32)
    for n in range(N):
        yr = res.results[n]["y"]
        q = yr[:, :HWi].reshape(O, NT, 128).astype(np.float32)
        sc = np.ascontiguousarray(yr[:, HWi:]).view(np.float32).reshape(O, NT, 1)
        out[n] = (q * sc).reshape(O, H, W)
    return out

